# revision 1
# baseline (speedup 1.0000x reference)
"""Mixtral-style sparse MoE block on 8 Trainium2 NeuronCores.

Strategy: expert-parallel. The router (tiny: T x H @ H x E) runs on the host
as part of input sharding; each core is assigned one expert and receives the
tokens routed to it (padded up to the token-slice grid), transposed to
feature-major layout. The heavy FFN compute (top-2 of 8 experts =
~720 GFLOP) runs on the cores as fp32r matmuls (11-bit-mantissa RNE, full
PE rate for free dims >= 256). The host applies the renormalized top-2
combine weights and scatter-adds the per-expert outputs back into the full
[T, H] output.

Per-core kernel math (C = token capacity, padded):
    h  = silu(x @ w1) * (x @ w3)    # [C, F]
    y  = h @ w2                     # [C, H]
computed in feature-on-partition layout: activations are [feature, token]
so all three weight matrices are used in their natural [K, M] layout as
matmul stationary operands and the SwiGLU intermediate h lands directly in
the [F-partition, token-free] layout that the down-projection consumes.

Token capacity = n full 512-token slices plus an optional 256/384-token
tail slice, so the padding waste on the max-loaded expert stays under one
half slice. Weights are streamed from HBM every slice (w1/w3) which, with
the x/y slices, keeps per-slice DMA (~33-47 MB) just under the PE's
~143 us/slice of matmul work.

W2_RESIDENT_BF16 variant: w2 is converted to bf16 on the host, loaded into
SBUF once, and the SwiGLU intermediate h is written as bf16, making the
down-projection a bf16 matmul. This halves the weight-DMA pressure and
removes the per-slice w2 reload at a small accuracy cost.
"""

import numpy as np

H = 1024        # hidden dim
F = 3584        # FFN dim
E = 8           # experts == cores
NT = 512        # full token slice (psum bank = 512 fp32)
KH = H // 128   # 8 k-tiles over hidden
MF = F // 128   # 28 m-tiles over ffn
MH = H // 128   # 8 m-tiles over hidden (down-proj output)

# MODE: "f32r" = all matmuls fp32r (most accurate);
#       "w2bf16" = w2 resident in bf16, h bf16, mm1/mm3 f32r;
#       "allbf16" = every matmul operand bf16 (least DMA + PE energy)
MODE = "f32r"
W2_RESIDENT_BF16 = MODE == "w2bf16"

_compile_cache = {}
_last_result = None  # BassKernelResults of the most recent run (for profiling)


def _slice_plan(max_cnt):
    """Token-slice widths covering max_cnt: full 512s + one >=256 tail."""
    n_full, rem = divmod(max(max_cnt, 256), NT)
    if rem == 0:
        tail = 0
    elif rem <= 256:
        tail = 256
    elif rem <= 384:
        tail = 384
    else:
        tail = NT
    return (NT,) * n_full + ((tail,) if tail else ())


def _build(slices):
    """Build + compile the per-core Bass program for the given slice widths."""
    import concourse.bass as bass
    import concourse.mybir as mybir
    import concourse.tile as tile
    from concourse import bacc

    C = sum(slices)
    f32 = mybir.dt.float32
    f32r = mybir.dt.float32r
    bf16 = mybir.dt.bfloat16
    x_dt = bf16 if MODE == "allbf16" else f32r
    w13_dt = bf16 if MODE == "allbf16" else f32r
    h_dt = f32r if MODE == "f32r" else bf16
    w2_dt = f32r if MODE == "f32r" else bf16
    ts = bass.ts

    nc = bacc.Bacc("TRN2", target_bir_lowering=False, debug=False, num_devices=E)

    xT = nc.dram_tensor("xT", [H, C], x_dt, kind="ExternalInput").ap()
    w1s = nc.dram_tensor("w1s", [MF, 128, H], w13_dt, kind="ExternalInput").ap()
    w3s = nc.dram_tensor("w3s", [MF, 128, H], w13_dt, kind="ExternalInput").ap()
    w2s = nc.dram_tensor("w2s", [MH, 128, F], w2_dt, kind="ExternalInput").ap()
    yT = nc.dram_tensor("yT", [H, C], f32, kind="ExternalOutput").ap()

    xT_r = xT.rearrange("(k p) t -> k p t", p=128)
    yT_r = yT.rearrange("(m p) t -> m p t", p=128)

    with tile.TileContext(nc, trace_sim=False) as tc:
        with (
            tc.tile_pool(name="xp", bufs=2 if MODE == "f32r" else 3) as xp,
            tc.tile_pool(name="w1p", bufs=4 if MODE == "f32r" else 6) as w1p,
            tc.tile_pool(name="w3p", bufs=4 if MODE == "f32r" else 6) as w3p,
            tc.tile_pool(name="w2p", bufs=MH if W2_RESIDENT_BF16 else 2) as w2p,
            tc.tile_pool(name="hp", bufs=MF + 6) as hp,
            tc.tile_pool(name="hsp", bufs=3) as hsp,
            tc.tile_pool(name="yp", bufs=3) as yp,
            tc.tile_pool(name="ps1p", bufs=3, space="PSUM") as ps1p,
            tc.tile_pool(name="ps3p", bufs=3, space="PSUM") as ps3p,
            tc.tile_pool(name="psyp", bufs=2, space="PSUM") as psyp,
        ):
            w2_res = None
            if W2_RESIDENT_BF16:
                w2_res = []
                for mh in range(MH):
                    w2t = w2p.tile([128, F], w2_dt)
                    nc.sync.dma_start(w2t[:], w2s[mh])
                    w2_res.append(w2t)

            offs = []
            off = 0
            for width in slices:
                offs.append((off, width))
                off += width
            for off, width in offs:
                xt = xp.tile([128, KH, NT], x_dt)
                for k in range(KH):
                    nc.sync.dma_start(
                        xt[:, k, :width], xT_r[k, :, off : off + width]
                    )

                h_tiles = []
                for m in range(MF):
                    w1t = w1p.tile([128, H], w13_dt)
                    nc.sync.dma_start(w1t[:], w1s[m])
                    w3t = w3p.tile([128, H], w13_dt)
                    nc.sync.dma_start(w3t[:], w3s[m])

                    ps1 = ps1p.tile([128, NT], f32)
                    for k in range(KH):
                        nc.tensor.matmul(
                            ps1[:, :width],
                            w1t[:, ts(k, 128)],
                            xt[:, k, :width],
                            start=(k == 0),
                            stop=(k == KH - 1),
                        )
                    ps3 = ps3p.tile([128, NT], f32)
                    for k in range(KH):
                        nc.tensor.matmul(
                            ps3[:, :width],
                            w3t[:, ts(k, 128)],
                            xt[:, k, :width],
                            start=(k == 0),
                            stop=(k == KH - 1),
                        )
                    hs = hsp.tile([128, NT], f32)
                    nc.scalar.activation(
                        hs[:, :width], ps1[:, :width],
                        mybir.ActivationFunctionType.Silu,
                    )
                    ht = hp.tile([128, NT], h_dt)
                    nc.vector.tensor_mul(ht[:, :width], hs[:, :width], ps3[:, :width])
                    h_tiles.append(ht)

                for mh in range(MH):
                    if W2_RESIDENT_BF16:
                        w2t = w2_res[mh]
                    else:
                        w2t = w2p.tile([128, F], w2_dt)
                        nc.sync.dma_start(w2t[:], w2s[mh])
                    psy = psyp.tile([128, NT], f32)
                    for kf in range(MF):
                        nc.tensor.matmul(
                            psy[:, :width],
                            w2t[:, ts(kf, 128)],
                            h_tiles[kf][:, :width],
                            start=(kf == 0),
                            stop=(kf == MF - 1),
                        )
                    yt = yp.tile([128, NT], f32)
                    nc.vector.tensor_copy(yt[:, :width], psy[:, :width])
                    nc.sync.dma_start(yT_r[mh, :, off : off + width], yt[:, :width])

    nc.compile()
    return nc


def _route(x, gate_w, gate_b):
    """Host router: top-2 expert ids + renormalized combine weights."""
    logits = x.astype(np.float32) @ gate_w.astype(np.float32).T + gate_b.astype(
        np.float32
    )
    # top-2 by prob == top-2 by logit (softmax is monotonic); stable sort
    # matches jax.lax.top_k's lower-index-first tie-breaking.
    top2 = np.argsort(-logits, axis=-1, kind="stable")[:, :2]
    l2 = np.take_along_axis(logits, top2, axis=1)
    e2 = np.exp(l2 - l2.max(axis=1, keepdims=True))
    wts = e2 / e2.sum(axis=1, keepdims=True)
    return top2, wts.astype(np.float32)


def kernel(x, gate_w, gate_b, w1, w3, w2):
    from concourse.bass_utils import run_bass_kernel_spmd

    x = np.asarray(x, dtype=np.float32)
    T = x.shape[0]
    top2, wts = _route(x, np.asarray(gate_w), np.asarray(gate_b))

    idx_list, scale_list = [], []
    for e in range(E):
        sel = top2 == e                      # [T, 2] bool
        tok = np.nonzero(sel.any(axis=1))[0]
        idx_list.append(tok)
        # each token picks an expert at most once, so this take is unique
        which = sel[tok, 1].astype(np.int64)  # 0 if slot0, 1 if slot1
        scale_list.append(wts[tok, which])

    max_cnt = max(len(i) for i in idx_list)
    slices = _slice_plan(max_cnt)
    C = sum(slices)

    nc = _compile_cache.get(slices)
    if nc is None:
        nc = _build(slices)
        _compile_cache[slices] = nc

    w1 = np.asarray(w1, dtype=np.float32)
    w3 = np.asarray(w3, dtype=np.float32)
    w2 = np.asarray(w2, dtype=np.float32)

    in_maps = []
    for e in range(E):
        tok = idx_list[e]
        xTe = np.zeros((H, C), np.float32)
        xTe[:, : len(tok)] = x[tok].T
        # W[k*128+p, m*128+c] -> [m, p, k*128+c]: 2KB-contiguous lhsT tiles
        w1s_e = np.ascontiguousarray(
            w1[e].reshape(KH, 128, MF, 128).transpose(2, 1, 0, 3).reshape(MF, 128, H)
        )
        w3s_e = np.ascontiguousarray(
            w3[e].reshape(KH, 128, MF, 128).transpose(2, 1, 0, 3).reshape(MF, 128, H)
        )
        w2s_e = np.ascontiguousarray(
            w2[e].reshape(MF, 128, MH, 128).transpose(2, 1, 0, 3).reshape(MH, 128, F)
        )
        if MODE != "f32r":
            import ml_dtypes

            w2s_e = w2s_e.astype(ml_dtypes.bfloat16)
        if MODE == "allbf16":
            import ml_dtypes

            xTe = xTe.astype(ml_dtypes.bfloat16)
            w1s_e = w1s_e.astype(ml_dtypes.bfloat16)
            w3s_e = w3s_e.astype(ml_dtypes.bfloat16)
        in_maps.append({"xT": xTe, "w1s": w1s_e, "w3s": w3s_e, "w2s": w2s_e})

    global _last_result
    res = run_bass_kernel_spmd(nc, in_maps, core_ids=list(range(E)))
    _last_result = res

    out = np.zeros((T, H), np.float32)
    for e in range(E):
        tok = idx_list[e]
        if len(tok) == 0:
            continue
        yTe = res.results[e]["yT"]
        out[tok] += yTe[:, : len(tok)].T * scale_list[e][:, None]
    return out



# revision 2
# speedup vs baseline: 1.1879x; 1.1879x over previous
"""Mixtral-style sparse MoE block on 8 Trainium2 NeuronCores.

Strategy: expert-parallel. The router (tiny: T x H @ H x E) runs on the host
as part of input sharding; each core is assigned one expert and receives the
tokens routed to it (padded up to the token-slice grid), transposed to
feature-major layout. The heavy FFN compute (top-2 of 8 experts =
~720 GFLOP) runs on the cores as fp32r matmuls (11-bit-mantissa RNE, full
PE rate for free dims >= 256). The host applies the renormalized top-2
combine weights and scatter-adds the per-expert outputs back into the full
[T, H] output.

Per-core kernel math (C = token capacity, padded):
    h  = silu(x @ w1) * (x @ w3)    # [C, F]
    y  = h @ w2                     # [C, H]
computed in feature-on-partition layout: activations are [feature, token]
so all three weight matrices are used in their natural [K, M] layout as
matmul stationary operands and the SwiGLU intermediate h lands directly in
the [F-partition, token-free] layout that the down-projection consumes.

Token capacity = n full 512-token slices plus an optional 256/384-token
tail slice, so the padding waste on the max-loaded expert stays under one
half slice. Weights are streamed from HBM every slice (w1/w3) which, with
the x/y slices, keeps per-slice DMA (~33-47 MB) just under the PE's
~143 us/slice of matmul work.

W2_RESIDENT_BF16 variant: w2 is converted to bf16 on the host, loaded into
SBUF once, and the SwiGLU intermediate h is written as bf16, making the
down-projection a bf16 matmul. This halves the weight-DMA pressure and
removes the per-slice w2 reload at a small accuracy cost.
"""

import numpy as np

H = 1024        # hidden dim
F = 3584        # FFN dim
E = 8           # experts == cores
NT = 512        # full token slice (psum bank = 512 fp32)
KH = H // 128   # 8 k-tiles over hidden
MF = F // 128   # 28 m-tiles over ffn
MH = H // 128   # 8 m-tiles over hidden (down-proj output)

# MODE: "f32r" = all matmuls fp32r (most accurate);
#       "w2bf16" = w2 resident in bf16, h bf16, mm1/mm3 f32r;
#       "allbf16" = every matmul operand bf16 (least DMA + PE energy)
MODE = "allbf16"
W2_RESIDENT_BF16 = MODE == "w2bf16"

_compile_cache = {}
_last_result = None  # BassKernelResults of the most recent run (for profiling)


def _slice_plan(max_cnt):
    """Token-slice widths covering max_cnt: full 512s + one >=256 tail."""
    n_full, rem = divmod(max(max_cnt, 256), NT)
    if rem == 0:
        tail = 0
    elif rem <= 256:
        tail = 256
    elif rem <= 384:
        tail = 384
    else:
        tail = NT
    return (NT,) * n_full + ((tail,) if tail else ())


def _build(slices):
    """Build + compile the per-core Bass program for the given slice widths."""
    import concourse.bass as bass
    import concourse.mybir as mybir
    import concourse.tile as tile
    from concourse import bacc

    C = sum(slices)
    f32 = mybir.dt.float32
    f32r = mybir.dt.float32r
    bf16 = mybir.dt.bfloat16
    x_dt = bf16 if MODE == "allbf16" else f32r
    w13_dt = bf16 if MODE == "allbf16" else f32r
    h_dt = f32r if MODE == "f32r" else bf16
    w2_dt = f32r if MODE == "f32r" else bf16
    ts = bass.ts

    nc = bacc.Bacc("TRN2", target_bir_lowering=False, debug=False, num_devices=E)

    xT = nc.dram_tensor("xT", [H, C], x_dt, kind="ExternalInput").ap()
    w1s = nc.dram_tensor("w1s", [MF, 128, H], w13_dt, kind="ExternalInput").ap()
    w3s = nc.dram_tensor("w3s", [MF, 128, H], w13_dt, kind="ExternalInput").ap()
    w2s = nc.dram_tensor("w2s", [MH, 128, F], w2_dt, kind="ExternalInput").ap()
    yT = nc.dram_tensor("yT", [H, C], f32, kind="ExternalOutput").ap()

    xT_r = xT.rearrange("(k p) t -> k p t", p=128)
    yT_r = yT.rearrange("(m p) t -> m p t", p=128)

    with tile.TileContext(nc, trace_sim=False) as tc:
        with (
            tc.tile_pool(name="xp", bufs=2 if MODE == "f32r" else 3) as xp,
            tc.tile_pool(name="w1p", bufs=4 if MODE == "f32r" else 6) as w1p,
            tc.tile_pool(name="w3p", bufs=4 if MODE == "f32r" else 6) as w3p,
            tc.tile_pool(name="w2p", bufs=MH if W2_RESIDENT_BF16 else 2) as w2p,
            tc.tile_pool(name="hp", bufs=MF + 6) as hp,
            tc.tile_pool(name="hsp", bufs=3) as hsp,
            tc.tile_pool(name="yp", bufs=3) as yp,
            tc.tile_pool(name="ps1p", bufs=3, space="PSUM") as ps1p,
            tc.tile_pool(name="ps3p", bufs=3, space="PSUM") as ps3p,
            tc.tile_pool(name="psyp", bufs=2, space="PSUM") as psyp,
        ):
            w2_res = None
            if W2_RESIDENT_BF16:
                w2_res = []
                for mh in range(MH):
                    w2t = w2p.tile([128, F], w2_dt)
                    nc.sync.dma_start(w2t[:], w2s[mh])
                    w2_res.append(w2t)

            offs = []
            off = 0
            for width in slices:
                offs.append((off, width))
                off += width
            for off, width in offs:
                xt = xp.tile([128, KH, NT], x_dt)
                for k in range(KH):
                    nc.sync.dma_start(
                        xt[:, k, :width], xT_r[k, :, off : off + width]
                    )

                h_tiles = []
                for m in range(MF):
                    w1t = w1p.tile([128, H], w13_dt)
                    nc.sync.dma_start(w1t[:], w1s[m])
                    w3t = w3p.tile([128, H], w13_dt)
                    nc.sync.dma_start(w3t[:], w3s[m])

                    ps1 = ps1p.tile([128, NT], f32)
                    for k in range(KH):
                        nc.tensor.matmul(
                            ps1[:, :width],
                            w1t[:, ts(k, 128)],
                            xt[:, k, :width],
                            start=(k == 0),
                            stop=(k == KH - 1),
                        )
                    ps3 = ps3p.tile([128, NT], f32)
                    for k in range(KH):
                        nc.tensor.matmul(
                            ps3[:, :width],
                            w3t[:, ts(k, 128)],
                            xt[:, k, :width],
                            start=(k == 0),
                            stop=(k == KH - 1),
                        )
                    hs = hsp.tile([128, NT], f32)
                    nc.scalar.activation(
                        hs[:, :width], ps1[:, :width],
                        mybir.ActivationFunctionType.Silu,
                    )
                    ht = hp.tile([128, NT], h_dt)
                    nc.vector.tensor_mul(ht[:, :width], hs[:, :width], ps3[:, :width])
                    h_tiles.append(ht)

                for mh in range(MH):
                    if W2_RESIDENT_BF16:
                        w2t = w2_res[mh]
                    else:
                        w2t = w2p.tile([128, F], w2_dt)
                        nc.sync.dma_start(w2t[:], w2s[mh])
                    psy = psyp.tile([128, NT], f32)
                    for kf in range(MF):
                        nc.tensor.matmul(
                            psy[:, :width],
                            w2t[:, ts(kf, 128)],
                            h_tiles[kf][:, :width],
                            start=(kf == 0),
                            stop=(kf == MF - 1),
                        )
                    yt = yp.tile([128, NT], f32)
                    nc.vector.tensor_copy(yt[:, :width], psy[:, :width])
                    nc.sync.dma_start(yT_r[mh, :, off : off + width], yt[:, :width])

    nc.compile()
    return nc


def _route(x, gate_w, gate_b):
    """Host router: top-2 expert ids + renormalized combine weights."""
    logits = x.astype(np.float32) @ gate_w.astype(np.float32).T + gate_b.astype(
        np.float32
    )
    # top-2 by prob == top-2 by logit (softmax is monotonic); stable sort
    # matches jax.lax.top_k's lower-index-first tie-breaking.
    top2 = np.argsort(-logits, axis=-1, kind="stable")[:, :2]
    l2 = np.take_along_axis(logits, top2, axis=1)
    e2 = np.exp(l2 - l2.max(axis=1, keepdims=True))
    wts = e2 / e2.sum(axis=1, keepdims=True)
    return top2, wts.astype(np.float32)


def kernel(x, gate_w, gate_b, w1, w3, w2):
    from concourse.bass_utils import run_bass_kernel_spmd

    x = np.asarray(x, dtype=np.float32)
    T = x.shape[0]
    top2, wts = _route(x, np.asarray(gate_w), np.asarray(gate_b))

    idx_list, scale_list = [], []
    for e in range(E):
        sel = top2 == e                      # [T, 2] bool
        tok = np.nonzero(sel.any(axis=1))[0]
        idx_list.append(tok)
        # each token picks an expert at most once, so this take is unique
        which = sel[tok, 1].astype(np.int64)  # 0 if slot0, 1 if slot1
        scale_list.append(wts[tok, which])

    max_cnt = max(len(i) for i in idx_list)
    slices = _slice_plan(max_cnt)
    C = sum(slices)

    nc = _compile_cache.get(slices)
    if nc is None:
        nc = _build(slices)
        _compile_cache[slices] = nc

    w1 = np.asarray(w1, dtype=np.float32)
    w3 = np.asarray(w3, dtype=np.float32)
    w2 = np.asarray(w2, dtype=np.float32)

    in_maps = []
    for e in range(E):
        tok = idx_list[e]
        xTe = np.zeros((H, C), np.float32)
        xTe[:, : len(tok)] = x[tok].T
        # W[k*128+p, m*128+c] -> [m, p, k*128+c]: 2KB-contiguous lhsT tiles
        w1s_e = np.ascontiguousarray(
            w1[e].reshape(KH, 128, MF, 128).transpose(2, 1, 0, 3).reshape(MF, 128, H)
        )
        w3s_e = np.ascontiguousarray(
            w3[e].reshape(KH, 128, MF, 128).transpose(2, 1, 0, 3).reshape(MF, 128, H)
        )
        w2s_e = np.ascontiguousarray(
            w2[e].reshape(MF, 128, MH, 128).transpose(2, 1, 0, 3).reshape(MH, 128, F)
        )
        if MODE != "f32r":
            import ml_dtypes

            w2s_e = w2s_e.astype(ml_dtypes.bfloat16)
        if MODE == "allbf16":
            import ml_dtypes

            xTe = xTe.astype(ml_dtypes.bfloat16)
            w1s_e = w1s_e.astype(ml_dtypes.bfloat16)
            w3s_e = w3s_e.astype(ml_dtypes.bfloat16)
        in_maps.append({"xT": xTe, "w1s": w1s_e, "w3s": w3s_e, "w2s": w2s_e})

    global _last_result
    res = run_bass_kernel_spmd(nc, in_maps, core_ids=list(range(E)))
    _last_result = res

    out = np.zeros((T, H), np.float32)
    for e in range(E):
        tok = idx_list[e]
        if len(tok) == 0:
            continue
        yTe = res.results[e]["yT"]
        out[tok] += yTe[:, : len(tok)].T * scale_list[e][:, None]
    return out



# revision 4
# speedup vs baseline: 1.2289x; 1.0346x over previous
"""Mixtral-style sparse MoE block on 8 Trainium2 NeuronCores.

Strategy: expert-parallel. The router (tiny: T x H @ H x E) runs on the host
as part of input sharding; each core is assigned one expert and receives the
tokens routed to it (padded up to the token-slice grid), transposed to
feature-major layout. The heavy FFN compute (top-2 of 8 experts =
~720 GFLOP) runs on the cores as fp32r matmuls (11-bit-mantissa RNE, full
PE rate for free dims >= 256). The host applies the renormalized top-2
combine weights and scatter-adds the per-expert outputs back into the full
[T, H] output.

Per-core kernel math (C = token capacity, padded):
    h  = silu(x @ w1) * (x @ w3)    # [C, F]
    y  = h @ w2                     # [C, H]
computed in feature-on-partition layout: activations are [feature, token]
so all three weight matrices are used in their natural [K, M] layout as
matmul stationary operands and the SwiGLU intermediate h lands directly in
the [F-partition, token-free] layout that the down-projection consumes.

Token capacity = n full 512-token slices plus an optional 256/384-token
tail slice, so the padding waste on the max-loaded expert stays under one
half slice. Weights are streamed from HBM every slice (w1/w3) which, with
the x/y slices, keeps per-slice DMA (~33-47 MB) just under the PE's
~143 us/slice of matmul work.

W2_RESIDENT_BF16 variant: w2 is converted to bf16 on the host, loaded into
SBUF once, and the SwiGLU intermediate h is written as bf16, making the
down-projection a bf16 matmul. This halves the weight-DMA pressure and
removes the per-slice w2 reload at a small accuracy cost.
"""

import numpy as np

H = 1024        # hidden dim
F = 3584        # FFN dim
E = 8           # experts == cores
NT = 512        # full token slice (psum bank = 512 fp32)
KH = H // 128   # 8 k-tiles over hidden
MF = F // 128   # 28 m-tiles over ffn
MH = H // 128   # 8 m-tiles over hidden (down-proj output)

# MODE: "f32r" = all matmuls fp32r (most accurate);
#       "w2bf16" = w2 resident in bf16, h bf16, mm1/mm3 f32r;
#       "allbf16" = every matmul operand bf16 (least DMA + PE energy)
MODE = "allbf16"
W2_RESIDENT_BF16 = MODE == "w2bf16"

_compile_cache = {}
_last_result = None  # BassKernelResults of the most recent run (for profiling)


def _slice_plan(max_cnt):
    """Token-slice widths covering max_cnt: full 512s + one 64-granular tail."""
    n_full, rem = divmod(max(max_cnt, 64), NT)
    tail = -(-rem // 64) * 64
    return (NT,) * n_full + ((tail,) if tail else ())


def _build(slices):
    """Build + compile the per-core Bass program for the given slice widths."""
    import concourse.bass as bass
    import concourse.mybir as mybir
    import concourse.tile as tile
    from concourse import bacc

    C = sum(slices)
    f32 = mybir.dt.float32
    f32r = mybir.dt.float32r
    bf16 = mybir.dt.bfloat16
    x_dt = bf16 if MODE == "allbf16" else f32r
    w13_dt = bf16 if MODE == "allbf16" else f32r
    h_dt = f32r if MODE == "f32r" else bf16
    w2_dt = f32r if MODE == "f32r" else bf16
    ts = bass.ts

    nc = bacc.Bacc("TRN2", target_bir_lowering=False, debug=False, num_devices=E)

    xT = nc.dram_tensor("xT", [H, C], x_dt, kind="ExternalInput").ap()
    w1s = nc.dram_tensor("w1s", [MF, 128, H], w13_dt, kind="ExternalInput").ap()
    w3s = nc.dram_tensor("w3s", [MF, 128, H], w13_dt, kind="ExternalInput").ap()
    w2s = nc.dram_tensor("w2s", [MH, 128, F], w2_dt, kind="ExternalInput").ap()
    yT = nc.dram_tensor("yT", [H, C], f32, kind="ExternalOutput").ap()

    xT_r = xT.rearrange("(k p) t -> k p t", p=128)
    yT_r = yT.rearrange("(m p) t -> m p t", p=128)

    with tile.TileContext(nc, trace_sim=False) as tc:
        with (
            tc.tile_pool(name="xp", bufs=2 if MODE == "f32r" else 3) as xp,
            tc.tile_pool(name="w1p", bufs=4 if MODE == "f32r" else 12) as w1p,
            tc.tile_pool(name="w3p", bufs=4 if MODE == "f32r" else 12) as w3p,
            tc.tile_pool(name="w2p", bufs=MH if W2_RESIDENT_BF16 else 3) as w2p,
            tc.tile_pool(name="hp", bufs=MF + 6) as hp,
            tc.tile_pool(name="hsp", bufs=3) as hsp,
            tc.tile_pool(name="yp", bufs=3) as yp,
            tc.tile_pool(name="ps1p", bufs=3, space="PSUM") as ps1p,
            tc.tile_pool(name="ps3p", bufs=3, space="PSUM") as ps3p,
            tc.tile_pool(name="psyp", bufs=2, space="PSUM") as psyp,
        ):
            w2_res = None
            if W2_RESIDENT_BF16:
                w2_res = []
                for mh in range(MH):
                    w2t = w2p.tile([128, F], w2_dt)
                    nc.sync.dma_start(w2t[:], w2s[mh])
                    w2_res.append(w2t)

            offs = []
            off = 0
            for width in slices:
                offs.append((off, width))
                off += width
            for off, width in offs:
                xt = xp.tile([128, KH, NT], x_dt)
                for k in range(KH):
                    nc.sync.dma_start(
                        xt[:, k, :width], xT_r[k, :, off : off + width]
                    )

                h_tiles = []
                for m in range(MF):
                    w1t = w1p.tile([128, H], w13_dt)
                    nc.sync.dma_start(w1t[:], w1s[m])
                    w3t = w3p.tile([128, H], w13_dt)
                    nc.sync.dma_start(w3t[:], w3s[m])

                    ps1 = ps1p.tile([128, NT], f32)
                    for k in range(KH):
                        nc.tensor.matmul(
                            ps1[:, :width],
                            w1t[:, ts(k, 128)],
                            xt[:, k, :width],
                            start=(k == 0),
                            stop=(k == KH - 1),
                        )
                    ps3 = ps3p.tile([128, NT], f32)
                    for k in range(KH):
                        nc.tensor.matmul(
                            ps3[:, :width],
                            w3t[:, ts(k, 128)],
                            xt[:, k, :width],
                            start=(k == 0),
                            stop=(k == KH - 1),
                        )
                    hs = hsp.tile([128, NT], f32)
                    nc.scalar.activation(
                        hs[:, :width], ps1[:, :width],
                        mybir.ActivationFunctionType.Silu,
                    )
                    ht = hp.tile([128, NT], h_dt)
                    nc.vector.tensor_mul(ht[:, :width], hs[:, :width], ps3[:, :width])
                    h_tiles.append(ht)

                for mh in range(MH):
                    if W2_RESIDENT_BF16:
                        w2t = w2_res[mh]
                    else:
                        w2t = w2p.tile([128, F], w2_dt)
                        nc.sync.dma_start(w2t[:], w2s[mh])
                    psy = psyp.tile([128, NT], f32)
                    for kf in range(MF):
                        nc.tensor.matmul(
                            psy[:, :width],
                            w2t[:, ts(kf, 128)],
                            h_tiles[kf][:, :width],
                            start=(kf == 0),
                            stop=(kf == MF - 1),
                        )
                    yt = yp.tile([128, NT], f32)
                    nc.vector.tensor_copy(yt[:, :width], psy[:, :width])
                    nc.sync.dma_start(yT_r[mh, :, off : off + width], yt[:, :width])

    nc.compile()
    return nc


def _route(x, gate_w, gate_b):
    """Host router: top-2 expert ids + renormalized combine weights."""
    logits = x.astype(np.float32) @ gate_w.astype(np.float32).T + gate_b.astype(
        np.float32
    )
    # top-2 by prob == top-2 by logit (softmax is monotonic); stable sort
    # matches jax.lax.top_k's lower-index-first tie-breaking.
    top2 = np.argsort(-logits, axis=-1, kind="stable")[:, :2]
    l2 = np.take_along_axis(logits, top2, axis=1)
    e2 = np.exp(l2 - l2.max(axis=1, keepdims=True))
    wts = e2 / e2.sum(axis=1, keepdims=True)
    return top2, wts.astype(np.float32)


def kernel(x, gate_w, gate_b, w1, w3, w2):
    from concourse.bass_utils import run_bass_kernel_spmd

    x = np.asarray(x, dtype=np.float32)
    T = x.shape[0]
    top2, wts = _route(x, np.asarray(gate_w), np.asarray(gate_b))

    idx_list, scale_list = [], []
    for e in range(E):
        sel = top2 == e                      # [T, 2] bool
        tok = np.nonzero(sel.any(axis=1))[0]
        idx_list.append(tok)
        # each token picks an expert at most once, so this take is unique
        which = sel[tok, 1].astype(np.int64)  # 0 if slot0, 1 if slot1
        scale_list.append(wts[tok, which])

    max_cnt = max(len(i) for i in idx_list)
    slices = _slice_plan(max_cnt)
    C = sum(slices)

    nc = _compile_cache.get(slices)
    if nc is None:
        nc = _build(slices)
        _compile_cache[slices] = nc

    w1 = np.asarray(w1, dtype=np.float32)
    w3 = np.asarray(w3, dtype=np.float32)
    w2 = np.asarray(w2, dtype=np.float32)

    in_maps = []
    for e in range(E):
        tok = idx_list[e]
        xTe = np.zeros((H, C), np.float32)
        xTe[:, : len(tok)] = x[tok].T
        # W[k*128+p, m*128+c] -> [m, p, k*128+c]: 2KB-contiguous lhsT tiles
        w1s_e = np.ascontiguousarray(
            w1[e].reshape(KH, 128, MF, 128).transpose(2, 1, 0, 3).reshape(MF, 128, H)
        )
        w3s_e = np.ascontiguousarray(
            w3[e].reshape(KH, 128, MF, 128).transpose(2, 1, 0, 3).reshape(MF, 128, H)
        )
        w2s_e = np.ascontiguousarray(
            w2[e].reshape(MF, 128, MH, 128).transpose(2, 1, 0, 3).reshape(MH, 128, F)
        )
        if MODE != "f32r":
            import ml_dtypes

            w2s_e = w2s_e.astype(ml_dtypes.bfloat16)
        if MODE == "allbf16":
            import ml_dtypes

            xTe = xTe.astype(ml_dtypes.bfloat16)
            w1s_e = w1s_e.astype(ml_dtypes.bfloat16)
            w3s_e = w3s_e.astype(ml_dtypes.bfloat16)
        in_maps.append({"xT": xTe, "w1s": w1s_e, "w3s": w3s_e, "w2s": w2s_e})

    global _last_result
    res = run_bass_kernel_spmd(nc, in_maps, core_ids=list(range(E)))
    _last_result = res

    out = np.zeros((T, H), np.float32)
    for e in range(E):
        tok = idx_list[e]
        if len(tok) == 0:
            continue
        yTe = res.results[e]["yT"]
        out[tok] += yTe[:, : len(tok)].T * scale_list[e][:, None]
    return out

